# revision 55
# baseline (speedup 1.0000x reference)
"""Distributed Trainium2 kernel for qk-norm attention.

Reference computation (B=2, N=2048, C=768, H=12, D=64):
    qkv = x @ W_qkv; q,k,v split per head
    q = LN(q)*scale, k = LN(k)   (LN over head_dim, with gamma/beta)
    out = softmax(q k^T) v ; y = concat_heads(out) @ W_proj + b_proj

Sharding: 24 (batch, head) units -> 8 cores: core c handles batch c//4
and heads 3*(c%4) .. 3*(c%4)+2.  Each core computes a partial
projection y_partial = out_heads @ W_proj[rows]; the host sums the 4
partials per batch and adds b_proj.

Per-core device program:
  - host passes x[b]^T as bf16 [768, 2048] (xt), W slices as bf16
  - qk_nat = x @ W_qk (natural layout), LN stats over head_dim via
    bn_stats; apply (x-mu)*rstd via fused tensor_scalar -> bf16
  - PE-transpose chunks -> qT/kT [*, 2048]; gamma/beta folded into the
    PSUM->SBUF copy as per-partition scalars (tensor_scalar mult+add)
  - scores^T[k,q] = kT.T @ qT per (head, k-tile): psum [128, 2048]
  - exp on ACT: exp(0.125*s - 4) (softmax shift; no max-subtract needed
    since qk-normed scores are O(1)); split in 2 ops for bank pipelining
  - AV: out^T[d,q] += v_aug[k, d|1].T @ expT[k,q]; the ones column
    yields softmax row sums in out^T row 64
  - 1/rowsum via reciprocal_approx_fast on a [4,512] reshape (DMA),
    broadcast across partitions with gpsimd.partition_broadcast
  - proj: y[tok,:] = out^T.T @ W_proj_rows, K=192 accumulation
"""

import contextlib
import sys

import numpy as np

sys.path.insert(0, "/opt/trn_rl_repo")

import ml_dtypes

import concourse.bass as bass
import concourse.tile as tile
from concourse import bacc, bass_utils, mybir
from concourse.masks import make_identity

BF16 = mybir.dt.bfloat16
F32 = mybir.dt.float32

B, N, C = 2, 2048, 768
H, D = 12, 64
HL = 3          # heads per core
P = 128
NT = N // P     # 16 token tiles
KC = C // P     # 6 contraction tiles over C
NQC = 4         # q chunks of 512
QC = 512
EPS = 1e-5
EXP_SHIFT = -4.0
SCALE = D ** -0.5  # 0.125


def _build(nc, apply_gb):
    """Emit the per-core program (SPMD: all 8 cores run this graph).

    apply_gb: emit gamma/beta fixup ops (skipped when gamma==1, beta==0,
    which is what the reference's setup_inputs produces).
    """
    xt_d = nc.dram_tensor("xt", [C, N], BF16, kind="ExternalInput")
    # 582 = 384 qk cols + 6 per-group mean cols + 192 v cols
    wqkv_d = nc.dram_tensor("wqkv", [C, 582], BF16, kind="ExternalInput")
    wp_d = nc.dram_tensor("wp", [HL * D, C], BF16, kind="ExternalInput")
    gb_d = nc.dram_tensor("gb", [12, P], F32, kind="ExternalInput")
    out_d = nc.dram_tensor("out", [N, C], F32, kind="ExternalOutput")

    with tile.TileContext(nc) as tc:
        ctx = contextlib.ExitStack()
        with ctx:
            singles = ctx.enter_context(tc.tile_pool(name="singles", bufs=1))
            persist = ctx.enter_context(tc.tile_pool(name="persist", bufs=1))

            # ---- constants ----
            ident = singles.tile([P, P], BF16)
            make_identity(nc, ident)
            eps_t = singles.tile([P, 1], F32)
            nc.vector.memset(eps_t, EPS)
            shift_t = singles.tile([P, 1], F32)
            nc.vector.memset(shift_t, EXP_SHIFT)
            zero_t = singles.tile([P, 1], F32)
            nc.vector.memset(zero_t, 0.0)
            gb_sb = singles.tile([P, 12], F32)
            nc.sync.dma_start(out=gb_sb, in_=gb_d.ap().rearrange("g p -> p g"))

            # ---- weights / x^T (split per k-tile so the first qkv
            # matmul can start as soon as slice 0 lands) ----
            wqkv_sb = persist.tile([P, KC, 582], BF16)
            xt_sb = persist.tile([P, KC, N], BF16)
            for kc in range(KC):
                ksl = slice(kc * P, (kc + 1) * P)
                nc.sync.dma_start(out=wqkv_sb[:, kc, :], in_=wqkv_d.ap()[ksl, :])
                nc.sync.dma_start(out=xt_sb[:, kc, :], in_=xt_d.ap()[ksl, :])
            wpA = persist.tile([P, C], BF16)
            nc.sync.dma_start(out=wpA, in_=wp_d.ap()[0:P, :])
            wpB = persist.tile([64, C], BF16)
            nc.sync.dma_start(out=wpB, in_=wp_d.ap()[P : P + 64, :])

            # ---- persistent activations ----
            qTA = persist.tile([P, N], BF16, tag="qTA")   # q0 | q1
            kTA = persist.tile([P, N], BF16, tag="kTA")   # k0 | k1
            # q2/k2 transposed via DMA xbar need 128-col sources; rows
            # 64-127 are junk from the zero padding and never read
            q2T = persist.tile([P, N], BF16, tag="q2T")
            k2T = persist.tile([P, N], BF16, tag="k2T")
            # v with ones column: [p, kt, h, 66] (cols 0-63 v, col 64 ones)
            v_all = persist.tile([P, NT, HL, 66], BF16, tag="v_all")
            nc.gpsimd.memset(v_all[:, :, :, 64:65], 1.0)
            # normalized out^T (proj lhsT): oTA = h0 | h1, oTB = h2
            oTA = persist.tile([P, N], BF16, tag="oTA")
            oTB = persist.tile([64, N], BF16, tag="oTB")

            # staging for deferred q2/k2 transposes (head 2); layout per mt:
            # [q2(64) | zeros(64) | k2(64) | zeros(64)] so each DMA-xbar
            # transpose source is a full [128, 128]
            qk_ln2 = persist.tile([P, NT, 256], BF16, tag="qk_ln2")
            nc.gpsimd.memset(qk_ln2, 0.0)

            # ============ phase 1a: qkv matmul + LN + q01/k01 transpose ====
            # W col order: [q0 q1 | k0 k1 | q2 | k2] then [v0 v1 v2]
            with tc.tile_pool(name="p1ps", bufs=3, space="PSUM") as pp1, \
                 tc.tile_pool(name="p1vps", bufs=2, space="PSUM") as pp1v, \
                 tc.tile_pool(name="p1tp", bufs=2, space="PSUM") as ppt, \
                 tc.tile_pool(name="p1sb", bufs=3) as ps1, \
                 tc.tile_pool(name="p1sq", bufs=2) as psq, \
                 tc.tile_pool(name="p1st", bufs=4) as pst:
                lag = []

                def _emit_transposes(mt, qk_ln):
                    msl = slice(mt * P, (mt + 1) * P)
                    for ch, (dst, gcol) in enumerate(((qTA, 0), (kTA, 1))):
                        tp = ppt.tile([P, P], BF16, tag="tp")
                        nc.tensor.transpose(
                            tp, qk_ln[:, ch * P : (ch + 1) * P], ident
                        )
                        if apply_gb:
                            nc.vector.tensor_scalar(
                                dst[:, msl], tp,
                                gb_sb[:, gcol : gcol + 1],
                                gb_sb[:, 6 + gcol : 7 + gcol],
                                op0=mybir.AluOpType.mult,
                                op1=mybir.AluOpType.add,
                            )
                        elif ch == 0:
                            nc.vector.tensor_copy(dst[:, msl], tp)
                        else:
                            # balance DVE (1a pacer) vs ScalarE
                            nc.scalar.copy(dst[:, msl], tp)

                for mt in range(NT):
                    msl = slice(mt * P, (mt + 1) * P)
                    qk_ps = pp1.tile([P, 390], F32, tag="qk_ps")
                    v_ps = pp1v.tile([P, 192], F32, tag="v_ps")
                    for kc in range(KC):
                        lhsT = xt_sb[:, kc, msl]
                        nc.tensor.matmul(
                            qk_ps, lhsT, wqkv_sb[:, kc, 0:390],
                            start=(kc == 0), stop=(kc == KC - 1),
                        )
                        nc.tensor.matmul(
                            v_ps, lhsT, wqkv_sb[:, kc, 390:582],
                            start=(kc == 0), stop=(kc == KC - 1),
                        )
                    nc.vector.tensor_copy(
                        v_all[:, mt, :, 0:64],
                        v_ps[:].rearrange("p (h d) -> p h d", h=HL),
                    )
                    # LN stats: mean comes from the 6 extra W columns (PE);
                    # E[x^2] via one whole-row Square + one grouped reduce
                    sq = psq.tile([P, 384], F32, tag="sq")
                    nc.scalar.activation(
                        sq, qk_ps[:, 0:384],
                        func=mybir.ActivationFunctionType.Square,
                        bias=zero_t,
                    )
                    ss = pst.tile([P, 6], F32, tag="ss")
                    nc.vector.tensor_reduce(
                        ss, sq[:].rearrange("p (g d) -> p g d", g=6),
                        axis=mybir.AxisListType.X, op=mybir.AluOpType.add,
                    )
                    mu = pst.tile([P, 6], F32, tag="mu")
                    nc.vector.tensor_copy(mu, qk_ps[:, 384:390])
                    rstd = pst.tile([P, 6], F32, tag="rstd")
                    # var = ss/64 - mu^2
                    nc.vector.tensor_mul(rstd, mu, mu)
                    nc.vector.scalar_tensor_tensor(
                        rstd, ss, 1.0 / 64, rstd,
                        op0=mybir.AluOpType.mult, op1=mybir.AluOpType.subtract,
                    )
                    nc.scalar.activation(
                        rstd, rstd,
                        func=mybir.ActivationFunctionType.Sqrt,
                        bias=eps_t, scale=1.0,
                    )
                    nc.vector.reciprocal(rstd, rstd)
                    nmr = pst.tile([P, 6], F32, tag="nmr")
                    nc.vector.scalar_tensor_tensor(
                        nmr, mu, -1.0, rstd,
                        op0=mybir.AluOpType.mult, op1=mybir.AluOpType.mult,
                    )
                    qk_ln = ps1.tile([P, 256], BF16, tag="qk_ln")
                    for g in range(6):
                        dst = (
                            qk_ln[:, g * 64 : (g + 1) * 64]
                            if g < 4
                            else qk_ln2[:, mt, (g - 4) * 128 : (g - 4) * 128 + 64]
                        )
                        if g % 2 == 0:
                            nc.vector.tensor_scalar(
                                dst, qk_ps[:, g * 64 : (g + 1) * 64],
                                mu[:, g : g + 1], rstd[:, g : g + 1],
                                op0=mybir.AluOpType.subtract,
                                op1=mybir.AluOpType.mult,
                            )
                        else:
                            # balance: odd groups applied on ScalarE as
                            # x*rstd + (-mu*rstd); nmr slice built below
                            nc.scalar.activation(
                                dst, qk_ps[:, g * 64 : (g + 1) * 64],
                                func=mybir.ActivationFunctionType.Identity,
                                bias=nmr[:, g : g + 1],
                                scale=rstd[:, g : g + 1],
                            )
                    # warm filler (K=128 so the HAM activity monitor sees a
                    # fully-active array; it drops the PE to 1.2 GHz when
                    # under-occupied and barely ever re-warms)
                    nwarm = 6 if mt == 0 else 1
                    for j in range(nwarm):
                        warmj = pp1v.tile([P, QC], F32, tag="v_ps")
                        nc.tensor.matmul(
                            warmj, xt_sb[:, 0, 0:P], wqkv_sb[:, 0, 0:QC],
                            start=True, stop=True,
                        )
                    # transposes run one mt behind so their LN inputs are
                    # ready by the time the in-order PE reaches them
                    lag.append((mt, qk_ln))
                    if len(lag) > 1:
                        _emit_transposes(*lag.pop(0))
                # cover the final LN chain latency, then flush the last
                # transposes
                for j in range(16):
                    warmj = pp1v.tile([P, QC], F32, tag="v_ps")
                    nc.tensor.matmul(
                        warmj, xt_sb[:, 0, 0:P], wqkv_sb[:, 0, 0:QC],
                        start=True, stop=True,
                    )
                _emit_transposes(*lag.pop(0))

            # ====== phase 1b (emitted early; runs on DMA during attn) ======
            for mt in range(NT):
                msl = slice(mt * P, (mt + 1) * P)
                nc.sync.dma_start_transpose(q2T[:, msl], qk_ln2[:, mt, 0:128])
                nc.sync.dma_start_transpose(k2T[:, msl], qk_ln2[:, mt, 128:256])
                if apply_gb:
                    for dst, gcol in ((q2T, 2), (k2T, 3)):
                        nc.vector.tensor_scalar(
                            dst[0:64, msl], dst[0:64, msl],
                            gb_sb[0:64, gcol : gcol + 1],
                            gb_sb[0:64, 6 + gcol : 7 + gcol],
                            op0=mybir.AluOpType.mult,
                            op1=mybir.AluOpType.add,
                        )

            # ================= phase 2: attention =================
            head_src = [(kTA, qTA, 0), (kTA, qTA, 64), (k2T, q2T, 0)]

            with tc.tile_pool(name="scps", bufs=2, space="PSUM") as psc, \
                 tc.tile_pool(name="avps", bufs=1, space="PSUM") as pav, \
                 tc.tile_pool(name="expsb", bufs=4) as pexp, \
                 tc.tile_pool(name="avfsb", bufs=2) as pavf, \
                 tc.tile_pool(name="sumsb", bufs=2) as psb:
                # The HAM clock gate drops the PE to 1.2 GHz whenever it is
                # not ~saturated, and with ACT (exp) pacing the attention it
                # would then never re-warm.  So (a) the kt loop is software-
                # pipelined (scores for kt+1 + filler run during exp(kt)),
                # and (b) filler matmuls top PE utilization up to the exp
                # pace.  Filler reads qTA blocks written late in phase 1a so
                # it unlocks progressively while the LN tail drains.
                def sc_mms(psc, kT, qT, r0, kt, half):
                    sct = psc.tile([P, 2 * QC], F32, tag="sc")
                    for q2 in range(2):
                        qc = 2 * half + q2
                        nc.tensor.matmul(
                            sct[:, q2 * QC : (q2 + 1) * QC],
                            kT[r0 : r0 + 64, kt * P : (kt + 1) * P],
                            qT[r0 : r0 + 64, qc * QC : (qc + 1) * QC],
                            start=True, stop=True,
                        )
                    return sct

                warm = psc.tile([P, 2 * QC], F32, tag="sc")
                for j in range(24):
                    blk = (12 + j // 6) % NT
                    nc.tensor.matmul(
                        warm[:, 0:P],
                        xt_sb[:, 0, 0:P],
                        qTA[:, blk * P : (blk + 1) * P],
                        start=True, stop=True,
                    )
                for h in range(HL):
                    kT, qT, r0 = head_src[h]
                    av_ps = pav.tile([65, N], F32, tag="av")
                    s0 = sc_mms(psc, kT, qT, r0, 0, 0)
                    s1 = sc_mms(psc, kT, qT, r0, 0, 1)
                    for kt in range(NT):
                        eT = pexp.tile([P, N], BF16, tag="expT")
                        nc.scalar.activation(
                            eT[:, 0:1024], s0,
                            func=mybir.ActivationFunctionType.Exp,
                            bias=shift_t, scale=SCALE,
                        )
                        nc.scalar.activation(
                            eT[:, 1024:2048], s1,
                            func=mybir.ActivationFunctionType.Exp,
                            bias=shift_t, scale=SCALE,
                        )
                        # filler into the old sc tile (runs during exp);
                        # K=128 keeps the HAM activity monitor happy
                        nj = 5 if (h == 0 and kt < 4) else 2
                        for j in range(nj):
                            nc.tensor.matmul(
                                s0[:, 0:QC],
                                xt_sb[:, 0, 0:P], wqkv_sb[:, 0, 0:QC],
                                start=True, stop=True,
                            )
                        if kt < NT - 1:
                            s0n = sc_mms(psc, kT, qT, r0, kt + 1, 0)
                        for qc in (0, 1):
                            nc.tensor.matmul(
                                av_ps[:, qc * QC : (qc + 1) * QC],
                                v_all[:, kt, h, 0:65],
                                eT[:, qc * QC : (qc + 1) * QC],
                                start=(kt == 0), stop=(kt == NT - 1),
                            )
                        if kt < NT - 1:
                            s1n = sc_mms(psc, kT, qT, r0, kt + 1, 1)
                        for qc in (2, 3):
                            nc.tensor.matmul(
                                av_ps[:, qc * QC : (qc + 1) * QC],
                                v_all[:, kt, h, 0:65],
                                eT[:, qc * QC : (qc + 1) * QC],
                                start=(kt == 0), stop=(kt == NT - 1),
                            )
                        if kt < NT - 1:
                            s0, s1 = s0n, s1n
                    # bridge the AV-psum drain before the next head's AV
                    for j in range(18 if h == 2 else 10):
                        nc.tensor.matmul(
                            s1[:, 0:QC],
                            xt_sb[:, 0, 0:P], wqkv_sb[:, 0, 0:QC],
                            start=True, stop=True,
                        )
                    # drain AV psum (frees the 4 banks for the next head)
                    avf = pavf.tile([65, N], F32, tag="avf")
                    nc.vector.tensor_copy(avf, av_ps)
                    # 1/rowsum: [1,2048] -> [4,512] (DMA), recip, back
                    s4 = psb.tile([4, QC], F32, tag="s4")
                    nc.gpsimd.dma_start(out=s4, in_=avf[64:65, :])
                    r4 = psb.tile([4, QC], F32, tag="r4")
                    nc.vector.reciprocal_approx_fast(out=r4, in_=s4)
                    if h == 2:
                        # keep-warm matmuls gated on the recip result so
                        # they unlock in step with the normalize chain
                        echo = psb.tile([4, QC], BF16, tag="echo")
                        nc.vector.tensor_copy(echo, r4)
                        warmE = psc.tile([P, 2 * QC], F32, tag="sc")
                        for j in range(40):
                            if j == 0:
                                # gate the filler stream on the recip result
                                # (in-order PE: the rest queue behind it)
                                nc.tensor.matmul(
                                    warmE[:, 0:QC], echo[:, 0:P],
                                    echo[:, 0:QC], start=True, stop=True,
                                )
                            else:
                                nc.tensor.matmul(
                                    warmE[:, 0:QC], xt_sb[:, 0, 0:P],
                                    wqkv_sb[:, 0, 0:QC], start=True, stop=True,
                                )
                    r1 = psb.tile([1, N], F32, tag="r1")
                    nc.gpsimd.dma_start(out=r1, in_=r4)
                    rb = psb.tile([64, N], F32, tag="rb")
                    nc.gpsimd.partition_broadcast(rb, r1, channels=64)
                    if h == 0:
                        nc.vector.tensor_mul(oTA[0:64, :], avf[0:64, :], rb)
                    elif h == 1:
                        # DVE cannot shift partitions; write base-0 tmp
                        # then DMA into oTA rows 64-127
                        tmp = psb.tile([64, N], BF16, tag="o1tmp")
                        nc.vector.tensor_mul(tmp, avf[0:64, :], rb)
                        nc.sync.dma_start(out=oTA[64:P, :], in_=tmp)
                    else:
                        nc.vector.tensor_mul(oTB[0:64, :], avf[0:64, :], rb)

                # residual bridge into the projection
                warm2 = psc.tile([P, 2 * QC], F32, tag="sc")
                for j in range(16):
                    nc.tensor.matmul(
                        warm2[:, 0:QC],
                        kTA[0:64, 0:128], qTA[0:64, 0:QC],
                        start=True, stop=True,
                    )

            # ================= phase 3: projection =================
            with tc.tile_pool(name="pjps", bufs=2, space="PSUM") as ppj, \
                 tc.tile_pool(name="ysb", bufs=3) as py:
                for mt in range(NT):
                    msl = slice(mt * P, (mt + 1) * P)
                    y_ps = ppj.tile([P, C], F32, tag="y")
                    warmp = ppj.tile([P, 256], F32, tag="warmp")
                    nc.tensor.matmul(
                        warmp, xt_sb[:, 0, 0:P], wqkv_sb[:, 0, 0:256],
                        start=True, stop=True,
                    )
                    for n0, n1 in [(0, 512), (512, 768)]:
                        nc.tensor.matmul(
                            y_ps[:, n0:n1], oTA[:, msl], wpA[:, n0:n1],
                            start=True, stop=False,
                        )
                        nc.tensor.matmul(
                            y_ps[:, n0:n1], oTB[:, msl], wpB[:, n0:n1],
                            start=False, stop=True,
                        )
                    y_out = py.tile([P, C], F32, tag="y_out")
                    if mt % 2 == 0:
                        nc.vector.tensor_copy(y_out, y_ps)
                    else:
                        nc.scalar.copy(y_out, y_ps)
                    nc.sync.dma_start(out=out_d.ap()[msl, :], in_=y_out)

    nc.compile()
    return nc


_CACHED = {}


def _get_nc(apply_gb):
    key = ("nc", apply_gb)
    if key not in _CACHED:
        nc = bacc.Bacc("TRN2", target_bir_lowering=False, debug=False)
        _CACHED[key] = _build(nc, apply_gb)
    return _CACHED[key]


def _make_in_maps(inputs):
    x = np.asarray(inputs["x"], np.float32)
    wqkv = np.asarray(inputs["W_qkv"], np.float32)
    wproj = np.asarray(inputs["W_proj"], np.float32)
    qg = np.asarray(inputs["q_gamma"], np.float32)
    qb = np.asarray(inputs["q_beta"], np.float32)
    kg = np.asarray(inputs["k_gamma"], np.float32)
    kb = np.asarray(inputs["k_beta"], np.float32)

    bf = ml_dtypes.bfloat16
    w3 = wqkv.reshape(C, 3, H, D)
    zero = np.zeros(D, np.float32)
    in_maps = []
    for c in range(8):
        b = c // 4
        h0 = (c % 4) * HL
        wq = w3[:, 0, h0 : h0 + HL, :]  # [C, 3, D]
        wk = w3[:, 1, h0 : h0 + HL, :]
        wv = w3[:, 2, h0 : h0 + HL, :]
        # cols: q0 q1 k0 k1 q2 k2 | 6 group-mean cols | v0 v1 v2
        qk_part = np.concatenate(
            [wq[:, 0], wq[:, 1], wk[:, 0], wk[:, 1], wq[:, 2], wk[:, 2]],
            axis=1,
        )  # [C, 384]
        means = qk_part.reshape(C, 6, D).mean(axis=2)  # [C, 6]
        wcols = np.concatenate(
            [qk_part, means, wv[:, 0], wv[:, 1], wv[:, 2]], axis=1
        )
        gbm = np.zeros((12, P), np.float32)
        gbm[0] = np.concatenate([qg, qg]); gbm[6] = np.concatenate([qb, qb])
        gbm[1] = np.concatenate([kg, kg]); gbm[7] = np.concatenate([kb, kb])
        gbm[2] = np.concatenate([qg, zero]); gbm[8] = np.concatenate([qb, zero])
        gbm[3] = np.concatenate([kg, zero]); gbm[9] = np.concatenate([kb, zero])
        in_maps.append(
            {
                "xt": np.ascontiguousarray(x[b].T).astype(bf),
                "wqkv": np.ascontiguousarray(wcols).astype(bf),
                "wp": np.ascontiguousarray(
                    wproj[h0 * D : (h0 + HL) * D, :]
                ).astype(bf),
                "gb": gbm,
            }
        )
    return in_maps


def _gather(inputs, results):
    bproj = np.asarray(inputs["b_proj"], np.float32)
    y = np.zeros((B, N, C), np.float32)
    for c in range(8):
        y[c // 4] += np.asarray(results[c]["out"])
    y += bproj
    return y


def _install_profile_hook():
    """The agent image's antenv lacks axon_hooks; synthesize it so
    run_bass_kernel_spmd(trace=True) can NTFF-profile via ctypes."""
    import types

    if "antenv.axon_hooks" in sys.modules:
        return
    try:
        from trn_agent_boot.trn_boot import _ntff_profile_via_ctypes

        hook = _ntff_profile_via_ctypes("/opt/axon/libaxon_pjrt.so")
    except Exception:
        hook = None
    mod = types.ModuleType("antenv.axon_hooks")
    mod.get_axon_ntff_profile_hook = lambda: hook
    mod.set_axon_ntff_profile_hook = lambda h: None
    sys.modules["antenv.axon_hooks"] = mod
    # no S3 in this container: keep artifacts local
    bass_utils.upload_artifacts = lambda tmpdir: tmpdir


def _kernel_impl(inputs, trace=False, tmpdir=None):
    apply_gb = not (
        np.all(np.asarray(inputs["q_gamma"]) == 1.0)
        and np.all(np.asarray(inputs["k_gamma"]) == 1.0)
        and np.all(np.asarray(inputs["q_beta"]) == 0.0)
        and np.all(np.asarray(inputs["k_beta"]) == 0.0)
    )
    nc = _get_nc(apply_gb)
    in_maps = _make_in_maps(inputs)
    if trace:
        _install_profile_hook()
    res = bass_utils.run_bass_kernel_spmd(
        nc, in_maps, core_ids=list(range(8)), trace=trace, tmpdir=tmpdir
    )
    out = _gather(inputs, res.results)
    return out, res


def kernel(**inputs):
    out, _ = _kernel_impl(inputs)
    return out


def kernel_with_profile(**inputs):
    out, res = _kernel_impl(inputs, trace=True)
    return out, res


# revision 56
# speedup vs baseline: 1.2779x; 1.2779x over previous
"""Distributed Trainium2 kernel for qk-norm attention.

Reference computation (B=2, N=2048, C=768, H=12, D=64):
    qkv = x @ W_qkv; q,k,v split per head
    q = LN(q)*scale, k = LN(k)   (LN over head_dim, with gamma/beta)
    out = softmax(q k^T) v ; y = concat_heads(out) @ W_proj + b_proj

Sharding: 24 (batch, head) units -> 8 cores: core c handles batch c//4
and heads 3*(c%4) .. 3*(c%4)+2.  Each core computes a partial
projection y_partial = out_heads @ W_proj[rows]; the host sums the 4
partials per batch and adds b_proj.

Per-core device program:
  - host passes x[b]^T as bf16 [768, 2048] (xt), W slices as bf16
  - qk_nat = x @ W_qk (natural layout), LN stats over head_dim via
    bn_stats; apply (x-mu)*rstd via fused tensor_scalar -> bf16
  - PE-transpose chunks -> qT/kT [*, 2048]; gamma/beta folded into the
    PSUM->SBUF copy as per-partition scalars (tensor_scalar mult+add)
  - scores^T[k,q] = kT.T @ qT per (head, k-tile): psum [128, 2048]
  - exp on ACT: exp(0.125*s - 4) (softmax shift; no max-subtract needed
    since qk-normed scores are O(1)); split in 2 ops for bank pipelining
  - AV: out^T[d,q] += v_aug[k, d|1].T @ expT[k,q]; the ones column
    yields softmax row sums in out^T row 64
  - 1/rowsum via reciprocal_approx_fast on a [4,512] reshape (DMA),
    broadcast across partitions with gpsimd.partition_broadcast
  - proj: y[tok,:] = out^T.T @ W_proj_rows, K=192 accumulation
"""

import contextlib
import sys

import numpy as np

sys.path.insert(0, "/opt/trn_rl_repo")

import ml_dtypes

import concourse.bass as bass
import concourse.tile as tile
from concourse import bacc, bass_utils, mybir
from concourse.masks import make_identity

BF16 = mybir.dt.bfloat16
F32 = mybir.dt.float32

B, N, C = 2, 2048, 768
H, D = 12, 64
HL = 3          # heads per core
P = 128
NT = N // P     # 16 token tiles
KC = C // P     # 6 contraction tiles over C
NQC = 4         # q chunks of 512
QC = 512
EPS = 1e-5
EXP_SHIFT = -4.0
SCALE = D ** -0.5  # 0.125


def _build(nc, apply_gb):
    """Emit the per-core program (SPMD: all 8 cores run this graph).

    apply_gb: emit gamma/beta fixup ops (skipped when gamma==1, beta==0,
    which is what the reference's setup_inputs produces).
    """
    xt_d = nc.dram_tensor("xt", [C, N], BF16, kind="ExternalInput")
    # 582 = 384 qk cols + 6 per-group mean cols + 192 v cols
    wqkv_d = nc.dram_tensor("wqkv", [C, 582], BF16, kind="ExternalInput")
    wp_d = nc.dram_tensor("wp", [HL * D, C], BF16, kind="ExternalInput")
    gb_d = nc.dram_tensor("gb", [12, P], F32, kind="ExternalInput")
    out_d = nc.dram_tensor("out", [N, C], F32, kind="ExternalOutput")

    with tile.TileContext(nc) as tc:
        ctx = contextlib.ExitStack()
        with ctx:
            singles = ctx.enter_context(tc.tile_pool(name="singles", bufs=1))
            persist = ctx.enter_context(tc.tile_pool(name="persist", bufs=1))

            # ---- constants ----
            ident = singles.tile([P, P], BF16)
            make_identity(nc, ident)
            eps_t = singles.tile([P, 1], F32)
            nc.vector.memset(eps_t, EPS)
            shift_t = singles.tile([P, 1], F32)
            nc.vector.memset(shift_t, EXP_SHIFT)
            zero_t = singles.tile([P, 1], F32)
            nc.vector.memset(zero_t, 0.0)
            gb_sb = singles.tile([P, 12], F32)
            nc.sync.dma_start(out=gb_sb, in_=gb_d.ap().rearrange("g p -> p g"))

            # ---- weights / x^T (split per k-tile so the first qkv
            # matmul can start as soon as slice 0 lands) ----
            wqkv_sb = persist.tile([P, KC, 582], BF16)
            xt_sb = persist.tile([P, KC, N], BF16)
            for kc in range(KC):
                ksl = slice(kc * P, (kc + 1) * P)
                nc.sync.dma_start(out=wqkv_sb[:, kc, :], in_=wqkv_d.ap()[ksl, :])
                nc.sync.dma_start(out=xt_sb[:, kc, :], in_=xt_d.ap()[ksl, :])
            wpA = persist.tile([P, C], BF16)
            nc.sync.dma_start(out=wpA, in_=wp_d.ap()[0:P, :])
            wpB = persist.tile([64, C], BF16)
            nc.sync.dma_start(out=wpB, in_=wp_d.ap()[P : P + 64, :])

            # ---- persistent activations ----
            qTA = persist.tile([P, N], BF16, tag="qTA")   # q0 | q1
            kTA = persist.tile([P, N], BF16, tag="kTA")   # k0 | k1
            # q2/k2 transposed via DMA xbar need 128-col sources; rows
            # 64-127 are junk from the zero padding and never read
            q2T = persist.tile([P, N], BF16, tag="q2T")
            k2T = persist.tile([P, N], BF16, tag="k2T")
            # v with ones column: [p, kt, h, 66] (cols 0-63 v, col 64 ones)
            v_all = persist.tile([P, NT, HL, 66], BF16, tag="v_all")
            nc.gpsimd.memset(v_all[:, :, :, 64:65], 1.0)
            # normalized out^T (proj lhsT): oTA = h0 | h1, oTB = h2
            oTA = persist.tile([P, N], BF16, tag="oTA")
            oTB = persist.tile([64, N], BF16, tag="oTB")

            # staging for deferred q2/k2 transposes (head 2); layout per mt:
            # [q2(64) | zeros(64) | k2(64) | zeros(64)] so each DMA-xbar
            # transpose source is a full [128, 128]
            qk_ln2 = persist.tile([P, NT, 256], BF16, tag="qk_ln2")
            nc.gpsimd.memset(qk_ln2, 0.0)

            # ============ phase 1a: qkv matmul + LN + q01/k01 transpose ====
            # W col order: [q0 q1 | k0 k1 | q2 | k2] then [v0 v1 v2]
            with tc.tile_pool(name="p1ps", bufs=3, space="PSUM") as pp1, \
                 tc.tile_pool(name="p1vps", bufs=2, space="PSUM") as pp1v, \
                 tc.tile_pool(name="p1tp", bufs=2, space="PSUM") as ppt, \
                 tc.tile_pool(name="p1sb", bufs=3) as ps1, \
                 tc.tile_pool(name="p1sq", bufs=2) as psq, \
                 tc.tile_pool(name="p1st", bufs=4) as pst:
                lag = []

                def _emit_transposes(mt, qk_ln):
                    msl = slice(mt * P, (mt + 1) * P)
                    for ch, (dst, gcol) in enumerate(((qTA, 0), (kTA, 1))):
                        tp = ppt.tile([P, P], BF16, tag="tp")
                        nc.tensor.transpose(
                            tp, qk_ln[:, ch * P : (ch + 1) * P], ident
                        )
                        if apply_gb:
                            nc.vector.tensor_scalar(
                                dst[:, msl], tp,
                                gb_sb[:, gcol : gcol + 1],
                                gb_sb[:, 6 + gcol : 7 + gcol],
                                op0=mybir.AluOpType.mult,
                                op1=mybir.AluOpType.add,
                            )
                        elif ch == 0:
                            nc.vector.tensor_copy(dst[:, msl], tp)
                        else:
                            # balance DVE (1a pacer) vs ScalarE
                            nc.scalar.copy(dst[:, msl], tp)

                for mt in range(NT):
                    msl = slice(mt * P, (mt + 1) * P)
                    qk_ps = pp1.tile([P, 390], F32, tag="qk_ps")
                    v_ps = pp1v.tile([P, 192], F32, tag="v_ps")
                    for kc in range(KC):
                        lhsT = xt_sb[:, kc, msl]
                        nc.tensor.matmul(
                            qk_ps, lhsT, wqkv_sb[:, kc, 0:390],
                            start=(kc == 0), stop=(kc == KC - 1),
                        )
                        nc.tensor.matmul(
                            v_ps, lhsT, wqkv_sb[:, kc, 390:582],
                            start=(kc == 0), stop=(kc == KC - 1),
                        )
                    nc.vector.tensor_copy(
                        v_all[:, mt, :, 0:64],
                        v_ps[:].rearrange("p (h d) -> p h d", h=HL),
                    )
                    # LN stats: mean comes from the 6 extra W columns (PE);
                    # E[x^2] via one whole-row Square + one grouped reduce
                    sq = psq.tile([P, 384], F32, tag="sq")
                    nc.scalar.activation(
                        sq, qk_ps[:, 0:384],
                        func=mybir.ActivationFunctionType.Square,
                        bias=zero_t,
                    )
                    ss = pst.tile([P, 6], F32, tag="ss")
                    nc.vector.tensor_reduce(
                        ss, sq[:].rearrange("p (g d) -> p g d", g=6),
                        axis=mybir.AxisListType.X, op=mybir.AluOpType.add,
                    )
                    mu = pst.tile([P, 6], F32, tag="mu")
                    nc.vector.tensor_copy(mu, qk_ps[:, 384:390])
                    rstd = pst.tile([P, 6], F32, tag="rstd")
                    # var = ss/64 - mu^2
                    nc.vector.tensor_mul(rstd, mu, mu)
                    nc.vector.scalar_tensor_tensor(
                        rstd, ss, 1.0 / 64, rstd,
                        op0=mybir.AluOpType.mult, op1=mybir.AluOpType.subtract,
                    )
                    nc.scalar.activation(
                        rstd, rstd,
                        func=mybir.ActivationFunctionType.Sqrt,
                        bias=eps_t, scale=1.0,
                    )
                    nc.vector.reciprocal(rstd, rstd)
                    nmr = pst.tile([P, 6], F32, tag="nmr")
                    nc.vector.scalar_tensor_tensor(
                        nmr, mu, -1.0, rstd,
                        op0=mybir.AluOpType.mult, op1=mybir.AluOpType.mult,
                    )
                    qk_ln = ps1.tile([P, 256], BF16, tag="qk_ln")
                    for g in range(6):
                        dst = (
                            qk_ln[:, g * 64 : (g + 1) * 64]
                            if g < 4
                            else qk_ln2[:, mt, (g - 4) * 128 : (g - 4) * 128 + 64]
                        )
                        if g % 2 == 0:
                            nc.vector.tensor_scalar(
                                dst, qk_ps[:, g * 64 : (g + 1) * 64],
                                mu[:, g : g + 1], rstd[:, g : g + 1],
                                op0=mybir.AluOpType.subtract,
                                op1=mybir.AluOpType.mult,
                            )
                        else:
                            # balance: odd groups applied on ScalarE as
                            # x*rstd + (-mu*rstd); nmr slice built below
                            nc.scalar.activation(
                                dst, qk_ps[:, g * 64 : (g + 1) * 64],
                                func=mybir.ActivationFunctionType.Identity,
                                bias=nmr[:, g : g + 1],
                                scale=rstd[:, g : g + 1],
                            )
                    # warm filler (K=128 so the HAM activity monitor sees a
                    # fully-active array; it drops the PE to 1.2 GHz when
                    # under-occupied and barely ever re-warms)
                    nwarm = 6 if mt == 0 else 1
                    for j in range(nwarm):
                        warmj = pp1v.tile([P, QC], F32, tag="v_ps")
                        nc.tensor.matmul(
                            warmj, xt_sb[:, 0, 0:P], wqkv_sb[:, 0, 0:QC],
                            start=True, stop=True,
                        )
                    # transposes run one mt behind so their LN inputs are
                    # ready by the time the in-order PE reaches them
                    lag.append((mt, qk_ln))
                    if len(lag) > 1:
                        _emit_transposes(*lag.pop(0))
                # cover the final LN chain latency, then flush the last
                # transposes
                for j in range(16):
                    warmj = pp1v.tile([P, QC], F32, tag="v_ps")
                    nc.tensor.matmul(
                        warmj, xt_sb[:, 0, 0:P], wqkv_sb[:, 0, 0:QC],
                        start=True, stop=True,
                    )
                _emit_transposes(*lag.pop(0))

            # ====== phase 1b (emitted early; runs on DMA during attn) ======
            for mt in range(NT):
                msl = slice(mt * P, (mt + 1) * P)
                nc.sync.dma_start_transpose(q2T[:, msl], qk_ln2[:, mt, 0:128])
                nc.sync.dma_start_transpose(k2T[:, msl], qk_ln2[:, mt, 128:256])
                if apply_gb:
                    for dst, gcol in ((q2T, 2), (k2T, 3)):
                        nc.vector.tensor_scalar(
                            dst[0:64, msl], dst[0:64, msl],
                            gb_sb[0:64, gcol : gcol + 1],
                            gb_sb[0:64, 6 + gcol : 7 + gcol],
                            op0=mybir.AluOpType.mult,
                            op1=mybir.AluOpType.add,
                        )

            # ================= phase 2: attention =================
            head_src = [(kTA, qTA, 0), (kTA, qTA, 64), (k2T, q2T, 0)]

            with tc.tile_pool(name="scps", bufs=2, space="PSUM") as psc, \
                 tc.tile_pool(name="avps", bufs=1, space="PSUM") as pav, \
                 tc.tile_pool(name="expsb", bufs=4) as pexp, \
                 tc.tile_pool(name="avfsb", bufs=2) as pavf, \
                 tc.tile_pool(name="sumsb", bufs=2) as psb:
                # The HAM clock gate drops the PE to 1.2 GHz whenever it is
                # not ~saturated, and with ACT (exp) pacing the attention it
                # would then never re-warm.  So (a) the kt loop is software-
                # pipelined (scores for kt+1 + filler run during exp(kt)),
                # and (b) filler matmuls top PE utilization up to the exp
                # pace.  Filler reads qTA blocks written late in phase 1a so
                # it unlocks progressively while the LN tail drains.
                def sc_mms(psc, kT, qT, r0, kt, half):
                    sct = psc.tile([P, 2 * QC], F32, tag="sc")
                    for q2 in range(2):
                        qc = 2 * half + q2
                        nc.tensor.matmul(
                            sct[:, q2 * QC : (q2 + 1) * QC],
                            kT[r0 : r0 + 64, kt * P : (kt + 1) * P],
                            qT[r0 : r0 + 64, qc * QC : (qc + 1) * QC],
                            start=True, stop=True,
                        )
                    return sct

                warm = psc.tile([P, 2 * QC], F32, tag="sc")
                for j in range(24):
                    blk = (12 + j // 6) % NT
                    nc.tensor.matmul(
                        warm[:, 0:P],
                        xt_sb[:, 0, 0:P],
                        qTA[:, blk * P : (blk + 1) * P],
                        start=True, stop=True,
                    )
                for h in range(HL):
                    kT, qT, r0 = head_src[h]
                    av_ps = pav.tile([65, N], F32, tag="av")
                    s0 = sc_mms(psc, kT, qT, r0, 0, 0)
                    s1 = sc_mms(psc, kT, qT, r0, 0, 1)
                    for kt in range(NT):
                        eT = pexp.tile([P, N], BF16, tag="expT")
                        nc.scalar.activation(
                            eT[:, 0:1024], s0,
                            func=mybir.ActivationFunctionType.Exp,
                            bias=shift_t, scale=SCALE,
                        )
                        nc.scalar.activation(
                            eT[:, 1024:2048], s1,
                            func=mybir.ActivationFunctionType.Exp,
                            bias=shift_t, scale=SCALE,
                        )
                        # filler into the old sc tile (runs during exp);
                        # K=128 keeps the HAM activity monitor happy
                        nj = 5 if (h == 0 and kt < 4) else 0
                        for j in range(nj):
                            nc.tensor.matmul(
                                s0[:, 0:QC],
                                xt_sb[:, 0, 0:P], wqkv_sb[:, 0, 0:QC],
                                start=True, stop=True,
                            )
                        if kt < NT - 1:
                            s0n = sc_mms(psc, kT, qT, r0, kt + 1, 0)
                        for qc in (0, 1):
                            nc.tensor.matmul(
                                av_ps[:, qc * QC : (qc + 1) * QC],
                                v_all[:, kt, h, 0:65],
                                eT[:, qc * QC : (qc + 1) * QC],
                                start=(kt == 0), stop=(kt == NT - 1),
                            )
                        if kt < NT - 1:
                            s1n = sc_mms(psc, kT, qT, r0, kt + 1, 1)
                        for qc in (2, 3):
                            nc.tensor.matmul(
                                av_ps[:, qc * QC : (qc + 1) * QC],
                                v_all[:, kt, h, 0:65],
                                eT[:, qc * QC : (qc + 1) * QC],
                                start=(kt == 0), stop=(kt == NT - 1),
                            )
                        if kt < NT - 1:
                            s0, s1 = s0n, s1n
                    # bridge the AV-psum drain before the next head's AV
                    for j in range(18 if h == 2 else 10):
                        nc.tensor.matmul(
                            s1[:, 0:QC],
                            xt_sb[:, 0, 0:P], wqkv_sb[:, 0, 0:QC],
                            start=True, stop=True,
                        )
                    # drain AV psum (frees the 4 banks for the next head)
                    avf = pavf.tile([65, N], F32, tag="avf")
                    nc.vector.tensor_copy(avf, av_ps)
                    # 1/rowsum: [1,2048] -> [4,512] (DMA), recip, back
                    s4 = psb.tile([4, QC], F32, tag="s4")
                    nc.gpsimd.dma_start(out=s4, in_=avf[64:65, :])
                    r4 = psb.tile([4, QC], F32, tag="r4")
                    nc.vector.reciprocal_approx_fast(out=r4, in_=s4)
                    if h == 2:
                        # keep-warm matmuls gated on the recip result so
                        # they unlock in step with the normalize chain
                        echo = psb.tile([4, QC], BF16, tag="echo")
                        nc.vector.tensor_copy(echo, r4)
                        warmE = psc.tile([P, 2 * QC], F32, tag="sc")
                        for j in range(40):
                            if j == 0:
                                # gate the filler stream on the recip result
                                # (in-order PE: the rest queue behind it)
                                nc.tensor.matmul(
                                    warmE[:, 0:QC], echo[:, 0:P],
                                    echo[:, 0:QC], start=True, stop=True,
                                )
                            else:
                                nc.tensor.matmul(
                                    warmE[:, 0:QC], xt_sb[:, 0, 0:P],
                                    wqkv_sb[:, 0, 0:QC], start=True, stop=True,
                                )
                    r1 = psb.tile([1, N], F32, tag="r1")
                    nc.gpsimd.dma_start(out=r1, in_=r4)
                    rb = psb.tile([64, N], F32, tag="rb")
                    nc.gpsimd.partition_broadcast(rb, r1, channels=64)
                    if h == 0:
                        nc.vector.tensor_mul(oTA[0:64, :], avf[0:64, :], rb)
                    elif h == 1:
                        # DVE cannot shift partitions; write base-0 tmp
                        # then DMA into oTA rows 64-127
                        tmp = psb.tile([64, N], BF16, tag="o1tmp")
                        nc.vector.tensor_mul(tmp, avf[0:64, :], rb)
                        nc.sync.dma_start(out=oTA[64:P, :], in_=tmp)
                    else:
                        nc.vector.tensor_mul(oTB[0:64, :], avf[0:64, :], rb)

                # residual bridge into the projection
                warm2 = psc.tile([P, 2 * QC], F32, tag="sc")
                for j in range(16):
                    nc.tensor.matmul(
                        warm2[:, 0:QC],
                        kTA[0:64, 0:128], qTA[0:64, 0:QC],
                        start=True, stop=True,
                    )

            # ================= phase 3: projection =================
            with tc.tile_pool(name="pjps", bufs=2, space="PSUM") as ppj, \
                 tc.tile_pool(name="ysb", bufs=3) as py:
                for mt in range(NT):
                    msl = slice(mt * P, (mt + 1) * P)
                    y_ps = ppj.tile([P, C], F32, tag="y")
                    warmp = ppj.tile([P, 256], F32, tag="warmp")
                    nc.tensor.matmul(
                        warmp, xt_sb[:, 0, 0:P], wqkv_sb[:, 0, 0:256],
                        start=True, stop=True,
                    )
                    for n0, n1 in [(0, 512), (512, 768)]:
                        nc.tensor.matmul(
                            y_ps[:, n0:n1], oTA[:, msl], wpA[:, n0:n1],
                            start=True, stop=False,
                        )
                        nc.tensor.matmul(
                            y_ps[:, n0:n1], oTB[:, msl], wpB[:, n0:n1],
                            start=False, stop=True,
                        )
                    y_out = py.tile([P, C], F32, tag="y_out")
                    if mt % 2 == 0:
                        nc.vector.tensor_copy(y_out, y_ps)
                    else:
                        nc.scalar.copy(y_out, y_ps)
                    nc.sync.dma_start(out=out_d.ap()[msl, :], in_=y_out)

    nc.compile()
    return nc


_CACHED = {}


def _get_nc(apply_gb):
    key = ("nc", apply_gb)
    if key not in _CACHED:
        nc = bacc.Bacc("TRN2", target_bir_lowering=False, debug=False)
        _CACHED[key] = _build(nc, apply_gb)
    return _CACHED[key]


def _make_in_maps(inputs):
    x = np.asarray(inputs["x"], np.float32)
    wqkv = np.asarray(inputs["W_qkv"], np.float32)
    wproj = np.asarray(inputs["W_proj"], np.float32)
    qg = np.asarray(inputs["q_gamma"], np.float32)
    qb = np.asarray(inputs["q_beta"], np.float32)
    kg = np.asarray(inputs["k_gamma"], np.float32)
    kb = np.asarray(inputs["k_beta"], np.float32)

    bf = ml_dtypes.bfloat16
    w3 = wqkv.reshape(C, 3, H, D)
    zero = np.zeros(D, np.float32)
    in_maps = []
    for c in range(8):
        b = c // 4
        h0 = (c % 4) * HL
        wq = w3[:, 0, h0 : h0 + HL, :]  # [C, 3, D]
        wk = w3[:, 1, h0 : h0 + HL, :]
        wv = w3[:, 2, h0 : h0 + HL, :]
        # cols: q0 q1 k0 k1 q2 k2 | 6 group-mean cols | v0 v1 v2
        qk_part = np.concatenate(
            [wq[:, 0], wq[:, 1], wk[:, 0], wk[:, 1], wq[:, 2], wk[:, 2]],
            axis=1,
        )  # [C, 384]
        means = qk_part.reshape(C, 6, D).mean(axis=2)  # [C, 6]
        wcols = np.concatenate(
            [qk_part, means, wv[:, 0], wv[:, 1], wv[:, 2]], axis=1
        )
        gbm = np.zeros((12, P), np.float32)
        gbm[0] = np.concatenate([qg, qg]); gbm[6] = np.concatenate([qb, qb])
        gbm[1] = np.concatenate([kg, kg]); gbm[7] = np.concatenate([kb, kb])
        gbm[2] = np.concatenate([qg, zero]); gbm[8] = np.concatenate([qb, zero])
        gbm[3] = np.concatenate([kg, zero]); gbm[9] = np.concatenate([kb, zero])
        in_maps.append(
            {
                "xt": np.ascontiguousarray(x[b].T).astype(bf),
                "wqkv": np.ascontiguousarray(wcols).astype(bf),
                "wp": np.ascontiguousarray(
                    wproj[h0 * D : (h0 + HL) * D, :]
                ).astype(bf),
                "gb": gbm,
            }
        )
    return in_maps


def _gather(inputs, results):
    bproj = np.asarray(inputs["b_proj"], np.float32)
    y = np.zeros((B, N, C), np.float32)
    for c in range(8):
        y[c // 4] += np.asarray(results[c]["out"])
    y += bproj
    return y


def _install_profile_hook():
    """The agent image's antenv lacks axon_hooks; synthesize it so
    run_bass_kernel_spmd(trace=True) can NTFF-profile via ctypes."""
    import types

    if "antenv.axon_hooks" in sys.modules:
        return
    try:
        from trn_agent_boot.trn_boot import _ntff_profile_via_ctypes

        hook = _ntff_profile_via_ctypes("/opt/axon/libaxon_pjrt.so")
    except Exception:
        hook = None
    mod = types.ModuleType("antenv.axon_hooks")
    mod.get_axon_ntff_profile_hook = lambda: hook
    mod.set_axon_ntff_profile_hook = lambda h: None
    sys.modules["antenv.axon_hooks"] = mod
    # no S3 in this container: keep artifacts local
    bass_utils.upload_artifacts = lambda tmpdir: tmpdir


def _kernel_impl(inputs, trace=False, tmpdir=None):
    apply_gb = not (
        np.all(np.asarray(inputs["q_gamma"]) == 1.0)
        and np.all(np.asarray(inputs["k_gamma"]) == 1.0)
        and np.all(np.asarray(inputs["q_beta"]) == 0.0)
        and np.all(np.asarray(inputs["k_beta"]) == 0.0)
    )
    nc = _get_nc(apply_gb)
    in_maps = _make_in_maps(inputs)
    if trace:
        _install_profile_hook()
    res = bass_utils.run_bass_kernel_spmd(
        nc, in_maps, core_ids=list(range(8)), trace=trace, tmpdir=tmpdir
    )
    out = _gather(inputs, res.results)
    return out, res


def kernel(**inputs):
    out, _ = _kernel_impl(inputs)
    return out


def kernel_with_profile(**inputs):
    out, res = _kernel_impl(inputs, trace=True)
    return out, res


# revision 60
# speedup vs baseline: 1.2865x; 1.0068x over previous
"""Distributed Trainium2 kernel for qk-norm attention.

Reference computation (B=2, N=2048, C=768, H=12, D=64):
    qkv = x @ W_qkv; q,k,v split per head
    q = LN(q)*scale, k = LN(k)   (LN over head_dim, with gamma/beta)
    out = softmax(q k^T) v ; y = concat_heads(out) @ W_proj + b_proj

Sharding: 24 (batch, head) units -> 8 cores: core c handles batch c//4
and heads 3*(c%4) .. 3*(c%4)+2.  Each core computes a partial
projection y_partial = out_heads @ W_proj[rows]; the host sums the 4
partials per batch and adds b_proj.

Per-core device program:
  - host passes x[b]^T as bf16 [768, 2048] (xt), W slices as bf16
  - qk_nat = x @ W_qk (natural layout), LN stats over head_dim via
    bn_stats; apply (x-mu)*rstd via fused tensor_scalar -> bf16
  - PE-transpose chunks -> qT/kT [*, 2048]; gamma/beta folded into the
    PSUM->SBUF copy as per-partition scalars (tensor_scalar mult+add)
  - scores^T[k,q] = kT.T @ qT per (head, k-tile): psum [128, 2048]
  - exp on ACT: exp(0.125*s - 4) (softmax shift; no max-subtract needed
    since qk-normed scores are O(1)); split in 2 ops for bank pipelining
  - AV: out^T[d,q] += v_aug[k, d|1].T @ expT[k,q]; the ones column
    yields softmax row sums in out^T row 64
  - 1/rowsum via reciprocal_approx_fast on a [4,512] reshape (DMA),
    broadcast across partitions with gpsimd.partition_broadcast
  - proj: y[tok,:] = out^T.T @ W_proj_rows, K=192 accumulation
"""

import contextlib
import sys

import numpy as np

sys.path.insert(0, "/opt/trn_rl_repo")

import ml_dtypes

import concourse.bass as bass
import concourse.tile as tile
from concourse import bacc, bass_utils, mybir
from concourse.masks import make_identity

BF16 = mybir.dt.bfloat16
F32 = mybir.dt.float32

B, N, C = 2, 2048, 768
H, D = 12, 64
HL = 3          # heads per core
P = 128
NT = N // P     # 16 token tiles
KC = C // P     # 6 contraction tiles over C
NQC = 4         # q chunks of 512
QC = 512
EPS = 1e-5
EXP_SHIFT = -4.0
SCALE = D ** -0.5  # 0.125


def _build(nc, apply_gb):
    """Emit the per-core program (SPMD: all 8 cores run this graph).

    apply_gb: emit gamma/beta fixup ops (skipped when gamma==1, beta==0,
    which is what the reference's setup_inputs produces).
    """
    xt_d = nc.dram_tensor("xt", [C, N], BF16, kind="ExternalInput")
    # 582 = 384 qk cols + 6 per-group mean cols + 192 v cols
    wqkv_d = nc.dram_tensor("wqkv", [C, 582], BF16, kind="ExternalInput")
    wp_d = nc.dram_tensor("wp", [HL * D, C], BF16, kind="ExternalInput")
    gb_d = nc.dram_tensor("gb", [12, P], F32, kind="ExternalInput")
    out_d = nc.dram_tensor("out", [N, C], F32, kind="ExternalOutput")

    with tile.TileContext(nc) as tc:
        ctx = contextlib.ExitStack()
        with ctx:
            singles = ctx.enter_context(tc.tile_pool(name="singles", bufs=1))
            persist = ctx.enter_context(tc.tile_pool(name="persist", bufs=1))

            # ---- constants ----
            ident = singles.tile([P, P], BF16)
            make_identity(nc, ident)
            eps_t = singles.tile([P, 1], F32)
            nc.vector.memset(eps_t, EPS)
            shift_t = singles.tile([P, 1], F32)
            nc.vector.memset(shift_t, EXP_SHIFT)
            zero_t = singles.tile([P, 1], F32)
            nc.vector.memset(zero_t, 0.0)
            gb_sb = singles.tile([P, 12], F32)
            nc.sync.dma_start(out=gb_sb, in_=gb_d.ap().rearrange("g p -> p g"))

            # ---- weights / x^T (split per k-tile so the first qkv
            # matmul can start as soon as slice 0 lands) ----
            wqkv_sb = persist.tile([P, KC, 582], BF16)
            xt_sb = persist.tile([P, KC, N], BF16)
            for kc in range(KC):
                ksl = slice(kc * P, (kc + 1) * P)
                nc.sync.dma_start(out=wqkv_sb[:, kc, :], in_=wqkv_d.ap()[ksl, :])
                nc.sync.dma_start(out=xt_sb[:, kc, :], in_=xt_d.ap()[ksl, :])
            wpA = persist.tile([P, C], BF16)
            nc.sync.dma_start(out=wpA, in_=wp_d.ap()[0:P, :])
            wpB = persist.tile([64, C], BF16)
            nc.sync.dma_start(out=wpB, in_=wp_d.ap()[P : P + 64, :])

            # ---- persistent activations ----
            qTA = persist.tile([P, N], BF16, tag="qTA")   # q0 | q1
            kTA = persist.tile([P, N], BF16, tag="kTA")   # k0 | k1
            # q2/k2 transposed via DMA xbar need 128-col sources; rows
            # 64-127 are junk from the zero padding and never read
            q2T = persist.tile([P, N], BF16, tag="q2T")
            k2T = persist.tile([P, N], BF16, tag="k2T")
            # v with ones column: [p, kt, h, 66] (cols 0-63 v, col 64 ones)
            v_all = persist.tile([P, NT, HL, 66], BF16, tag="v_all")
            nc.gpsimd.memset(v_all[:, :, :, 64:65], 1.0)
            # normalized out^T (proj lhsT): oTA = h0 | h1, oTB = h2
            oTA = persist.tile([P, N], BF16, tag="oTA")
            oTB = persist.tile([64, N], BF16, tag="oTB")

            # staging for deferred q2/k2 transposes (head 2); layout per mt:
            # [q2(64) | zeros(64) | k2(64) | zeros(64)] so each DMA-xbar
            # transpose source is a full [128, 128]
            qk_ln2 = persist.tile([P, NT, 256], BF16, tag="qk_ln2")
            nc.gpsimd.memset(qk_ln2, 0.0)

            # ============ phase 1a: qkv matmul + LN + q01/k01 transpose ====
            # W col order: [q0 q1 | k0 k1 | q2 | k2] then [v0 v1 v2]
            with tc.tile_pool(name="p1ps", bufs=3, space="PSUM") as pp1, \
                 tc.tile_pool(name="p1vps", bufs=2, space="PSUM") as pp1v, \
                 tc.tile_pool(name="p1tp", bufs=2, space="PSUM") as ppt, \
                 tc.tile_pool(name="p1sb", bufs=3) as ps1, \
                 tc.tile_pool(name="p1sq", bufs=2) as psq, \
                 tc.tile_pool(name="p1st", bufs=4) as pst:
                lag = []

                def _emit_transposes(mt, qk_ln):
                    msl = slice(mt * P, (mt + 1) * P)
                    for ch, (dst, gcol) in enumerate(((qTA, 0), (kTA, 1))):
                        tp = ppt.tile([P, P], BF16, tag="tp")
                        nc.tensor.transpose(
                            tp, qk_ln[:, ch * P : (ch + 1) * P], ident
                        )
                        if apply_gb:
                            nc.vector.tensor_scalar(
                                dst[:, msl], tp,
                                gb_sb[:, gcol : gcol + 1],
                                gb_sb[:, 6 + gcol : 7 + gcol],
                                op0=mybir.AluOpType.mult,
                                op1=mybir.AluOpType.add,
                            )
                        elif ch == 0:
                            nc.vector.tensor_copy(dst[:, msl], tp)
                        else:
                            # balance DVE (1a pacer) vs ScalarE
                            nc.scalar.copy(dst[:, msl], tp)

                for mt in range(NT):
                    msl = slice(mt * P, (mt + 1) * P)
                    qk_ps = pp1.tile([P, 390], F32, tag="qk_ps")
                    v_ps = pp1v.tile([P, 192], F32, tag="v_ps")
                    for kc in range(KC):
                        lhsT = xt_sb[:, kc, msl]
                        nc.tensor.matmul(
                            qk_ps, lhsT, wqkv_sb[:, kc, 0:390],
                            start=(kc == 0), stop=(kc == KC - 1),
                        )
                        nc.tensor.matmul(
                            v_ps, lhsT, wqkv_sb[:, kc, 390:582],
                            start=(kc == 0), stop=(kc == KC - 1),
                        )
                    nc.vector.tensor_copy(
                        v_all[:, mt, :, 0:64],
                        v_ps[:].rearrange("p (h d) -> p h d", h=HL),
                    )
                    # LN stats: mean comes from the 6 extra W columns (PE);
                    # E[x^2] via one whole-row Square + one grouped reduce
                    sq = psq.tile([P, 384], F32, tag="sq")
                    nc.scalar.activation(
                        sq, qk_ps[:, 0:384],
                        func=mybir.ActivationFunctionType.Square,
                        bias=zero_t,
                    )
                    ss = pst.tile([P, 6], F32, tag="ss")
                    nc.vector.tensor_reduce(
                        ss, sq[:].rearrange("p (g d) -> p g d", g=6),
                        axis=mybir.AxisListType.X, op=mybir.AluOpType.add,
                    )
                    mu = pst.tile([P, 6], F32, tag="mu")
                    nc.vector.tensor_copy(mu, qk_ps[:, 384:390])
                    rstd = pst.tile([P, 6], F32, tag="rstd")
                    # var = ss/64 - mu^2
                    nc.vector.tensor_mul(rstd, mu, mu)
                    nc.vector.scalar_tensor_tensor(
                        rstd, ss, 1.0 / 64, rstd,
                        op0=mybir.AluOpType.mult, op1=mybir.AluOpType.subtract,
                    )
                    nc.scalar.activation(
                        rstd, rstd,
                        func=mybir.ActivationFunctionType.Sqrt,
                        bias=eps_t, scale=1.0,
                    )
                    nc.vector.reciprocal(rstd, rstd)
                    nmr = pst.tile([P, 6], F32, tag="nmr")
                    nc.vector.scalar_tensor_tensor(
                        nmr, mu, -1.0, rstd,
                        op0=mybir.AluOpType.mult, op1=mybir.AluOpType.mult,
                    )
                    qk_ln = ps1.tile([P, 256], BF16, tag="qk_ln")
                    for g in range(6):
                        dst = (
                            qk_ln[:, g * 64 : (g + 1) * 64]
                            if g < 4
                            else qk_ln2[:, mt, (g - 4) * 128 : (g - 4) * 128 + 64]
                        )
                        if g % 2 == 0:
                            nc.vector.tensor_scalar(
                                dst, qk_ps[:, g * 64 : (g + 1) * 64],
                                mu[:, g : g + 1], rstd[:, g : g + 1],
                                op0=mybir.AluOpType.subtract,
                                op1=mybir.AluOpType.mult,
                            )
                        else:
                            # balance: odd groups applied on ScalarE as
                            # x*rstd + (-mu*rstd); nmr slice built below
                            nc.scalar.activation(
                                dst, qk_ps[:, g * 64 : (g + 1) * 64],
                                func=mybir.ActivationFunctionType.Identity,
                                bias=nmr[:, g : g + 1],
                                scale=rstd[:, g : g + 1],
                            )
                    # warm filler (K=128 so the HAM activity monitor sees a
                    # fully-active array; it drops the PE to 1.2 GHz when
                    # under-occupied and barely ever re-warms)
                    nwarm = 6 if mt == 0 else 1
                    for j in range(nwarm):
                        warmj = pp1v.tile([P, QC], F32, tag="v_ps")
                        nc.tensor.matmul(
                            warmj, xt_sb[:, 0, 0:P], wqkv_sb[:, 0, 0:QC],
                            start=True, stop=True,
                        )
                    # transposes run one mt behind so their LN inputs are
                    # ready by the time the in-order PE reaches them
                    lag.append((mt, qk_ln))
                    if len(lag) > 1:
                        _emit_transposes(*lag.pop(0))
                # cover the final LN chain latency, then flush the last
                # transposes
                for j in range(10):
                    warmj = pp1v.tile([P, QC], F32, tag="v_ps")
                    nc.tensor.matmul(
                        warmj, xt_sb[:, 0, 0:P], wqkv_sb[:, 0, 0:QC],
                        start=True, stop=True,
                    )
                # filler gated on the last LN applies ticks the PE along
                # the chain instead of front-running it
                qk15 = lag[0][1]
                for g in range(4):
                    warmg = pp1v.tile([P, QC], F32, tag="v_ps")
                    nc.tensor.matmul(
                        warmg[0:64, :], qk15[:, g * 64 : (g + 1) * 64],
                        wqkv_sb[:, 0, 0:QC], start=True, stop=True,
                    )
                _emit_transposes(*lag.pop(0))
                # pre-load the exp table set before phase 2 needs it
                primer = pst.tile([P, 1], F32, tag="primer")
                nc.scalar.activation(
                    primer, eps_t,
                    func=mybir.ActivationFunctionType.Exp,
                    bias=shift_t, scale=1.0,
                )

            # ====== phase 1b (emitted early; runs on DMA during attn) ======
            for mt in range(NT):
                msl = slice(mt * P, (mt + 1) * P)
                nc.sync.dma_start_transpose(q2T[:, msl], qk_ln2[:, mt, 0:128])
                nc.sync.dma_start_transpose(k2T[:, msl], qk_ln2[:, mt, 128:256])
                if apply_gb:
                    for dst, gcol in ((q2T, 2), (k2T, 3)):
                        nc.vector.tensor_scalar(
                            dst[0:64, msl], dst[0:64, msl],
                            gb_sb[0:64, gcol : gcol + 1],
                            gb_sb[0:64, 6 + gcol : 7 + gcol],
                            op0=mybir.AluOpType.mult,
                            op1=mybir.AluOpType.add,
                        )

            # ================= phase 2: attention =================
            head_src = [(kTA, qTA, 0), (kTA, qTA, 64), (k2T, q2T, 0)]

            with tc.tile_pool(name="scps", bufs=2, space="PSUM") as psc, \
                 tc.tile_pool(name="avps", bufs=1, space="PSUM") as pav, \
                 tc.tile_pool(name="expsb", bufs=4) as pexp, \
                 tc.tile_pool(name="avfsb", bufs=2) as pavf, \
                 tc.tile_pool(name="sumsb", bufs=2) as psb:
                # The HAM clock gate drops the PE to 1.2 GHz whenever it is
                # not ~saturated, and with ACT (exp) pacing the attention it
                # would then never re-warm.  So (a) the kt loop is software-
                # pipelined (scores for kt+1 + filler run during exp(kt)),
                # and (b) filler matmuls top PE utilization up to the exp
                # pace.  Filler reads qTA blocks written late in phase 1a so
                # it unlocks progressively while the LN tail drains.
                def sc_mms(psc, kT, qT, r0, kt, half):
                    sct = psc.tile([P, 2 * QC], F32, tag="sc")
                    for q2 in range(2):
                        qc = 2 * half + q2
                        nc.tensor.matmul(
                            sct[:, q2 * QC : (q2 + 1) * QC],
                            kT[r0 : r0 + 64, kt * P : (kt + 1) * P],
                            qT[r0 : r0 + 64, qc * QC : (qc + 1) * QC],
                            start=True, stop=True,
                        )
                    return sct

                warm = psc.tile([P, 2 * QC], F32, tag="sc")
                for j in range(24):
                    blk = (12 + j // 6) % NT
                    nc.tensor.matmul(
                        warm[:, 0:P],
                        xt_sb[:, 0, 0:P],
                        qTA[:, blk * P : (blk + 1) * P],
                        start=True, stop=True,
                    )
                for h in range(HL):
                    kT, qT, r0 = head_src[h]
                    av_ps = pav.tile([65, N], F32, tag="av")
                    s0 = sc_mms(psc, kT, qT, r0, 0, 0)
                    s1 = sc_mms(psc, kT, qT, r0, 0, 1)
                    for kt in range(NT):
                        eT = pexp.tile([P, N], BF16, tag="expT")
                        nc.scalar.activation(
                            eT[:, 0:1024], s0,
                            func=mybir.ActivationFunctionType.Exp,
                            bias=shift_t, scale=SCALE,
                        )
                        nc.scalar.activation(
                            eT[:, 1024:2048], s1,
                            func=mybir.ActivationFunctionType.Exp,
                            bias=shift_t, scale=SCALE,
                        )
                        # filler into the old sc tile (runs during exp);
                        # K=128 keeps the HAM activity monitor happy
                        nj = 5 if (h == 0 and kt < 4) else 0
                        for j in range(nj):
                            nc.tensor.matmul(
                                s0[:, 0:QC],
                                xt_sb[:, 0, 0:P], wqkv_sb[:, 0, 0:QC],
                                start=True, stop=True,
                            )
                        if kt < NT - 1:
                            s0n = sc_mms(psc, kT, qT, r0, kt + 1, 0)
                        for qc in (0, 1):
                            nc.tensor.matmul(
                                av_ps[:, qc * QC : (qc + 1) * QC],
                                v_all[:, kt, h, 0:65],
                                eT[:, qc * QC : (qc + 1) * QC],
                                start=(kt == 0), stop=(kt == NT - 1),
                            )
                        if kt < NT - 1:
                            s1n = sc_mms(psc, kT, qT, r0, kt + 1, 1)
                        for qc in (2, 3):
                            nc.tensor.matmul(
                                av_ps[:, qc * QC : (qc + 1) * QC],
                                v_all[:, kt, h, 0:65],
                                eT[:, qc * QC : (qc + 1) * QC],
                                start=(kt == 0), stop=(kt == NT - 1),
                            )
                        if kt < NT - 1:
                            s0, s1 = s0n, s1n
                    # bridge the AV-psum drain before the next head's AV
                    for j in range(22 if h == 2 else 10):
                        nc.tensor.matmul(
                            s1[:, 0:QC],
                            xt_sb[:, 0, 0:P], wqkv_sb[:, 0, 0:QC],
                            start=True, stop=True,
                        )
                    # drain AV psum (frees the 4 banks for the next head)
                    avf = pavf.tile([65, N], F32, tag="avf")
                    nc.vector.tensor_copy(avf, av_ps)
                    # 1/rowsum: [1,2048] -> [4,512] (DMA), recip, back
                    s4 = psb.tile([4, QC], F32, tag="s4")
                    nc.gpsimd.dma_start(out=s4, in_=avf[64:65, :])
                    r4 = psb.tile([4, QC], F32, tag="r4")
                    nc.vector.reciprocal_approx_fast(out=r4, in_=s4)
                    if h == 2:
                        # keep-warm matmuls gated on the recip result so
                        # they unlock in step with the normalize chain
                        echo = psb.tile([4, QC], BF16, tag="echo")
                        nc.vector.tensor_copy(echo, r4)
                        warmE = psc.tile([P, 2 * QC], F32, tag="sc")
                        for j in range(40):
                            if j == 0:
                                # gate the filler stream on the recip result
                                # (in-order PE: the rest queue behind it)
                                nc.tensor.matmul(
                                    warmE[:, 0:QC], echo[:, 0:P],
                                    echo[:, 0:QC], start=True, stop=True,
                                )
                            else:
                                nc.tensor.matmul(
                                    warmE[:, 0:QC], xt_sb[:, 0, 0:P],
                                    wqkv_sb[:, 0, 0:QC], start=True, stop=True,
                                )
                    r1 = psb.tile([1, N], F32, tag="r1")
                    nc.gpsimd.dma_start(out=r1, in_=r4)
                    rb = psb.tile([64, N], F32, tag="rb")
                    nc.gpsimd.partition_broadcast(rb, r1, channels=64)
                    if h == 0:
                        nc.vector.tensor_mul(oTA[0:64, :], avf[0:64, :], rb)
                    elif h == 1:
                        # DVE cannot shift partitions; write base-0 tmp
                        # then DMA into oTA rows 64-127
                        tmp = psb.tile([64, N], BF16, tag="o1tmp")
                        nc.vector.tensor_mul(tmp, avf[0:64, :], rb)
                        nc.sync.dma_start(out=oTA[64:P, :], in_=tmp)
                    else:
                        nc.vector.tensor_mul(oTB[0:64, :], avf[0:64, :], rb)

                # residual bridge into the projection
                warm2 = psc.tile([P, 2 * QC], F32, tag="sc")
                for j in range(16):
                    nc.tensor.matmul(
                        warm2[:, 0:QC],
                        kTA[0:64, 0:128], qTA[0:64, 0:QC],
                        start=True, stop=True,
                    )

            # ================= phase 3: projection =================
            with tc.tile_pool(name="pjps", bufs=3, space="PSUM") as ppj, \
                 tc.tile_pool(name="pjw", bufs=2, space="PSUM") as ppw, \
                 tc.tile_pool(name="ysb", bufs=4) as py:
                for mt in range(NT):
                    msl = slice(mt * P, (mt + 1) * P)
                    y_ps = ppj.tile([P, C], F32, tag="y")
                    warmp = ppw.tile([P, 128], F32, tag="warmp")
                    nc.tensor.matmul(
                        warmp, xt_sb[:, 0, 0:P], wqkv_sb[:, 0, 0:128],
                        start=True, stop=True,
                    )
                    for n0, n1 in [(0, 512), (512, 768)]:
                        nc.tensor.matmul(
                            y_ps[:, n0:n1], oTA[:, msl], wpA[:, n0:n1],
                            start=True, stop=False,
                        )
                        nc.tensor.matmul(
                            y_ps[:, n0:n1], oTB[:, msl], wpB[:, n0:n1],
                            start=False, stop=True,
                        )
                    y_out = py.tile([P, C], F32, tag="y_out")
                    # split the drain across both copy engines
                    nc.vector.tensor_copy(y_out[:, 0:384], y_ps[:, 0:384])
                    nc.scalar.copy(y_out[:, 384:768], y_ps[:, 384:768])
                    nc.sync.dma_start(out=out_d.ap()[msl, :], in_=y_out)

    nc.compile()
    return nc


_CACHED = {}


def _get_nc(apply_gb):
    key = ("nc", apply_gb)
    if key not in _CACHED:
        nc = bacc.Bacc("TRN2", target_bir_lowering=False, debug=False)
        _CACHED[key] = _build(nc, apply_gb)
    return _CACHED[key]


def _make_in_maps(inputs):
    x = np.asarray(inputs["x"], np.float32)
    wqkv = np.asarray(inputs["W_qkv"], np.float32)
    wproj = np.asarray(inputs["W_proj"], np.float32)
    qg = np.asarray(inputs["q_gamma"], np.float32)
    qb = np.asarray(inputs["q_beta"], np.float32)
    kg = np.asarray(inputs["k_gamma"], np.float32)
    kb = np.asarray(inputs["k_beta"], np.float32)

    bf = ml_dtypes.bfloat16
    w3 = wqkv.reshape(C, 3, H, D)
    zero = np.zeros(D, np.float32)
    in_maps = []
    for c in range(8):
        b = c // 4
        h0 = (c % 4) * HL
        wq = w3[:, 0, h0 : h0 + HL, :]  # [C, 3, D]
        wk = w3[:, 1, h0 : h0 + HL, :]
        wv = w3[:, 2, h0 : h0 + HL, :]
        # cols: q0 q1 k0 k1 q2 k2 | 6 group-mean cols | v0 v1 v2
        qk_part = np.concatenate(
            [wq[:, 0], wq[:, 1], wk[:, 0], wk[:, 1], wq[:, 2], wk[:, 2]],
            axis=1,
        )  # [C, 384]
        means = qk_part.reshape(C, 6, D).mean(axis=2)  # [C, 6]
        wcols = np.concatenate(
            [qk_part, means, wv[:, 0], wv[:, 1], wv[:, 2]], axis=1
        )
        gbm = np.zeros((12, P), np.float32)
        gbm[0] = np.concatenate([qg, qg]); gbm[6] = np.concatenate([qb, qb])
        gbm[1] = np.concatenate([kg, kg]); gbm[7] = np.concatenate([kb, kb])
        gbm[2] = np.concatenate([qg, zero]); gbm[8] = np.concatenate([qb, zero])
        gbm[3] = np.concatenate([kg, zero]); gbm[9] = np.concatenate([kb, zero])
        in_maps.append(
            {
                "xt": np.ascontiguousarray(x[b].T).astype(bf),
                "wqkv": np.ascontiguousarray(wcols).astype(bf),
                "wp": np.ascontiguousarray(
                    wproj[h0 * D : (h0 + HL) * D, :]
                ).astype(bf),
                "gb": gbm,
            }
        )
    return in_maps


def _gather(inputs, results):
    bproj = np.asarray(inputs["b_proj"], np.float32)
    y = np.zeros((B, N, C), np.float32)
    for c in range(8):
        y[c // 4] += np.asarray(results[c]["out"])
    y += bproj
    return y


def _install_profile_hook():
    """The agent image's antenv lacks axon_hooks; synthesize it so
    run_bass_kernel_spmd(trace=True) can NTFF-profile via ctypes."""
    import types

    if "antenv.axon_hooks" in sys.modules:
        return
    try:
        from trn_agent_boot.trn_boot import _ntff_profile_via_ctypes

        hook = _ntff_profile_via_ctypes("/opt/axon/libaxon_pjrt.so")
    except Exception:
        hook = None
    mod = types.ModuleType("antenv.axon_hooks")
    mod.get_axon_ntff_profile_hook = lambda: hook
    mod.set_axon_ntff_profile_hook = lambda h: None
    sys.modules["antenv.axon_hooks"] = mod
    # no S3 in this container: keep artifacts local
    bass_utils.upload_artifacts = lambda tmpdir: tmpdir


def _kernel_impl(inputs, trace=False, tmpdir=None):
    apply_gb = not (
        np.all(np.asarray(inputs["q_gamma"]) == 1.0)
        and np.all(np.asarray(inputs["k_gamma"]) == 1.0)
        and np.all(np.asarray(inputs["q_beta"]) == 0.0)
        and np.all(np.asarray(inputs["k_beta"]) == 0.0)
    )
    nc = _get_nc(apply_gb)
    in_maps = _make_in_maps(inputs)
    if trace:
        _install_profile_hook()
    res = bass_utils.run_bass_kernel_spmd(
        nc, in_maps, core_ids=list(range(8)), trace=trace, tmpdir=tmpdir
    )
    out = _gather(inputs, res.results)
    return out, res


def kernel(**inputs):
    out, _ = _kernel_impl(inputs)
    return out


def kernel_with_profile(**inputs):
    out, res = _kernel_impl(inputs, trace=True)
    return out, res
